# revision 40
# baseline (speedup 1.0000x reference)
"""Distributed Trainium2 Bass kernel for nn_AGCN (gnn_message_passing).

Reference computation (B=1, C=CHNN=1024, K=L=2048):
    vcw  = softmax_k(W_ak @ vc + b_ak)            # (K, L) assignment weights
    vmr  = relu(W_c @ vm + b_c)                   # (C, K)
    vma  = vmr @ vcw                              # (C, L)
    vmad = W_mad @ vma + b_mad                    # (C, L)
    A    = vmad^T @ vmad                          # (K, L) gram (symmetric)
    x    = vmr^T @ W_gcn + b_gcn                  # (K, C)
    out  = (softmax_rows(A) @ x)^T                # (C, L)

Distribution: position (node) sharding across 8 NeuronCores; core i owns
256 of the 2048 node columns.  Everything is local except three fp8
AllGathers on the serial ncfw stream:
  AG1  vmrT (2MB)  — right behind the kernel-entry barrier
  AGx  x = vmr^T W_gcn (2MB) — rides the stream's idle window between AG1
       and AG2, so it is effectively free; it eliminates the final W_gcn
       GEMM stage (out = (A_sm @ vmr^T) @ W_gcn == A_sm @ x, and the
       b_gcn fold is exact because softmax rows sum to 1)
  AG2  vmad shards in two k-chunks so the gram stage consumes chunk a
       while chunk b is in flight.

Matmuls are bf16/fp8 with fp32 PSUM accumulation (hw rel err ~1.9e-3 vs
the f32 reference; the A softmax is near-uniform so gram/fp8 noise
averages out).  Softmaxes skip max-subtraction (z in +-3.4, A in
[16.8, 17.2]); expA is scaled into fp8 range via a constant -12 bias
inside the ACT Exp (the colsum normalization auto-compensates).
Normalizations ride the PSUM-evacuation DVE ops; column sums are
computed on all partitions at once via ones-matrix matmuls with a lag-1
interleave so the PE never waits on ACT.  Dummy-matmul warm-keepers
(single long accumulation groups) bridge the collective windows to keep
the PE HAM clock at 2.4 GHz.  DMA emission order matters: loads gated on
collective semaphores are emitted after everything that must not queue
behind them.
"""

import numpy as np
import ml_dtypes

import concourse.bass as bass
import concourse.mybir as mybir
import concourse.tile as tile
from concourse import bacc
from concourse import bass_utils

P = 128            # partitions
C = 1024           # channels (8 tiles)
K = 2048           # nodes (16 tiles)
NCORES = 8
KL = K // NCORES   # 256 local node columns per core
KLH = KL // 2      # 128 (l/AG2 chunk width)
CT = C // P        # 8
KT = K // P        # 16
KLT = KL // P      # 2

BF = mybir.dt.bfloat16
F8 = mybir.dt.float8e4
F32 = mybir.dt.float32
RG = [list(range(NCORES))]

Exp = mybir.ActivationFunctionType.Exp
Identity = mybir.ActivationFunctionType.Identity

WARM0 = 100   # PE warm-keeper matmul counts (0 = disabled)
WARM1 = 320
WARM2 = 150
WARM3 = 60


def build():
    nc = bacc.Bacc("TRN2", target_bir_lowering=False, debug=False,
                   num_devices=NCORES)

    # ---- kernel I/O (per-core) ----
    vc_i = nc.dram_tensor("vc_i", [C, KL], BF, kind="ExternalInput").ap()
    vm_i = nc.dram_tensor("vm_i", [C, KL], BF, kind="ExternalInput").ap()
    W_akT = nc.dram_tensor("W_akT", [C, K], BF, kind="ExternalInput").ap()
    W_cT = nc.dram_tensor("W_cT", [C, C], BF, kind="ExternalInput").ap()
    W_madT = nc.dram_tensor("W_madT", [C, C], BF, kind="ExternalInput").ap()
    W_gcn = nc.dram_tensor("W_gcn", [C, C], BF, kind="ExternalInput").ap()
    b_ak_t = nc.dram_tensor("b_ak_t", [P, KT], F32, kind="ExternalInput").ap()
    b_cB = nc.dram_tensor("b_cB", [P, C], F32, kind="ExternalInput").ap()
    b_c_t = nc.dram_tensor("b_c_t", [P, CT], F32, kind="ExternalInput").ap()
    b_mad_t = nc.dram_tensor("b_mad_t", [P, CT], F32, kind="ExternalInput").ap()
    b_gcn_t = nc.dram_tensor("b_gcn_t", [P, CT], F32, kind="ExternalInput").ap()
    out = nc.dram_tensor("out", [C, KL], F32, kind="ExternalOutput").ap()

    with tile.TileContext(nc) as tc:
        with (
            tc.tile_pool(name="const", bufs=1) as const,
            tc.tile_pool(name="stage", bufs=4) as stage,
            tc.tile_pool(name="psum", bufs=8, space="PSUM") as pp,
            tc.tile_pool(name="dram", bufs=1, space="DRAM") as dram,
        ):
            # ---- persistent SBUF tensors ----
            vm_sb = const.tile([P, CT, KL], BF)       # vm   [p, ct, kl]
            vc_sb = const.tile([P, CT, KL], BF)
            WcT_sb = const.tile([P, CT, C], BF)
            WakT_sb = const.tile([P, CT, K], BF)
            WmadT_sb = const.tile([P, CT, C], BF)
            Wgcn_sb = const.tile([P, CT, C], BF)
            vmrT_full = const.tile([P, KT, C], F8)    # vmr^T[k global, c]
            vmr_ck = const.tile([P, CT, KL], BF)      # vmr  [c, k_loc]
            # x = vmr^T W_gcn gathered in fp8; shares W_gcn's slot (W_gcn is
            # only needed by the early x GEMM)
            x_full = const.tile([P, KT, C], F8, tag="Wgcn_sb")
            # vmad_full by AG2 chunk: A = even global k-tiles, B = odd;
            # [p, ct, s, kl] = vmad[ct*128+p, s*256 + q*128 + kl]
            # chunk A shares the WakT slot (dead after S3, disjoint lifetime)
            vmad_fullA = const.tile([P, CT, NCORES, KLH], F8, tag="WakT_sb")
            vmad_fullB = const.tile([P, CT, NCORES, KLH], F8)
            exp_sb = const.tile([P, KT, KL], F8)      # expz then expA (reused)
            expA_sb = exp_sb
            vma_sb = const.tile([P, CT, KL], BF)
            vmad_i_sb = const.tile([P, CT, KL], F8)
            b_ak_sb = const.tile([P, KT], F32)
            b_cB_sb = const.tile([P, C], F32)
            b_c_t_sb = const.tile([P, CT], F32)
            b_mad_sb = const.tile([P, CT], F32)
            b_gcn_sb = const.tile([P, CT], F32)
            onesm = const.tile([P, P], F8)
            negbias = const.tile([P, 1], F32)         # -12.0 for scaled expA
            recipL = const.tile([P, KL], F32)
            recipR = const.tile([P, KL], F32)

            nc.any.memset(onesm, 1.0)
            nc.any.memset(negbias, -12.0)

            # ---- front input loads: S1's deps only ----
            nc.sync.dma_start(out=b_cB_sb, in_=b_cB)
            for ct in range(CT):
                rows = slice(ct * P, (ct + 1) * P)
                nc.sync.dma_start(out=vm_sb[:, ct, :], in_=vm_i[rows, :])
            for ct in range(CT):
                rows = slice(ct * P, (ct + 1) * P)
                for h in range(2):
                    cols = slice(h * 512, (h + 1) * 512)
                    nc.sync.dma_start(out=WcT_sb[:, ct, cols],
                                      in_=W_cT[rows, cols])
            nc.sync.dma_start(out=b_c_t_sb, in_=b_c_t)
            nc.sync.dma_start(out=b_ak_sb, in_=b_ak_t)
            nc.sync.dma_start(out=b_mad_sb, in_=b_mad_t)
            nc.sync.dma_start(out=b_gcn_sb, in_=b_gcn_t)

            # ---- collective bounce buffers ----
            ag1_in = dram.tile([KL, C], F8)
            ag1_out = dram.tile([K, C], F8, addr_space="Shared")
            agx_in = dram.tile([KL, C], F8)
            agx_out = dram.tile([K, C], F8, addr_space="Shared")
            ag2a_in = dram.tile([C, KLH], F8)
            ag2a_out = dram.tile([NCORES, C, KLH], F8, addr_space="Shared")
            ag2b_in = dram.tile([C, KLH], F8)
            ag2b_out = dram.tile([NCORES, C, KLH], F8, addr_space="Shared")

            # ======= S1: vmrT_i = relu(vm_i^T W_c^T + b_c), (k_loc, c) =====
            with nc.named_scope("S1_vmrT"):
                for kt in range(KLT):
                    ksl = slice(kt * P, (kt + 1) * P)
                    for n in range(2):
                        nsl = slice(n * 512, (n + 1) * 512)
                        ps = pp.tile([P, 512], F32, tag="ps", name=f"ps1_{kt}_{n}")
                        for cc in range(CT):
                            nc.tensor.matmul(ps, vm_sb[:, cc, ksl],
                                             WcT_sb[:, cc, nsl],
                                             start=(cc == 0),
                                             stop=(cc == CT - 1))
                        tmp = stage.tile([P, 512], F32, tag="s1tmp")
                        nc.vector.tensor_add(tmp, ps, b_cB_sb[:, nsl])
                        relu = stage.tile([P, 512], F8, tag="s1relu")
                        nc.vector.tensor_scalar_max(relu, tmp, 0.0)
                        nc.sync.dma_start(out=ag1_in[ksl, nsl], in_=relu)

            # ======= AG1: all-gather vmrT (single op — chunking halves the
            # per-op bus bandwidth and nothing can fill a second window) ====
            nc.gpsimd.collective_compute(
                "AllGather", mybir.AluOpType.bypass, replica_groups=RG,
                ins=[ag1_in.opt()], outs=[ag1_out.opt()],
            )

            # W_gcn loads ride after the AG1 bounce chain so they don't
            # delay its trigger; S2b (their consumer) runs well before AGx
            for ct in range(CT):
                rows = slice(ct * P, (ct + 1) * P)
                for h in range(2):
                    cols = slice(h * 512, (h + 1) * 512)
                    nc.sync.dma_start(out=Wgcn_sb[:, ct, cols],
                                      in_=W_gcn[rows, cols])

            # ======= S1': vmr_i = relu(W_c vm_i + b_c), (c, k_loc) =========
            # feeds the early x GEMM; runs inside the entry-barrier window
            with nc.named_scope("S1p_vmr"):
                for m in range(CT):
                    msl = slice(m * P, (m + 1) * P)
                    ps = pp.tile([P, KL], F32, tag="ps", name=f"ps1p_{m}")
                    for cc in range(CT):
                        nc.tensor.matmul(ps, WcT_sb[:, cc, msl],
                                         vm_sb[:, cc, :],
                                         start=(cc == 0), stop=(cc == CT - 1))
                    nc.scalar.activation(vmr_ck[:, m, :], ps,
                                         mybir.ActivationFunctionType.Relu,
                                         bias=b_c_t_sb[:, m:m + 1], scale=1.0)

            # ======= S2b: x_i = vmr_i^T W_gcn, (k_loc, co) =================
            with nc.named_scope("S2b_x"):
                for kt in range(KLT):
                    ksl = slice(kt * P, (kt + 1) * P)
                    for n in range(2):
                        nsl = slice(n * 512, (n + 1) * 512)
                        ps = pp.tile([P, 512], F32, tag="ps",
                                     name=f"ps2b_{kt}_{n}")
                        for cc in range(CT):
                            nc.tensor.matmul(ps, vmr_ck[:, cc, ksl],
                                             Wgcn_sb[:, cc, nsl],
                                             start=(cc == 0),
                                             stop=(cc == CT - 1))
                        xs = stage.tile([P, 512], F8, tag="xstage")
                        nc.vector.tensor_copy(xs, ps)
                        nc.sync.dma_start(out=agx_in[ksl, nsl], in_=xs)

            # ======= AGx: all-gather x — rides the ncfw stream's idle
            # window between AG1 and AG2, so it is effectively free =========
            nc.gpsimd.collective_compute(
                "AllGather", mybir.AluOpType.bypass, replica_groups=RG,
                ins=[agx_in.opt()], outs=[agx_out.opt()],
            )

            # ---- remaining input loads (S3/S5 deps), after AG1's chain
            # (warm-keeper defined early so S1->S3 DMA wait stays warm)
            warm_scratch = const.tile([P, P], F32)

            def warm(n, label):
                dps = pp.tile([P, P], F32, tag="ps", name=f"warm_{label}")
                for i in range(n):
                    nc.tensor.matmul(dps, onesm, onesm,
                                     start=(i == 0), stop=(i == n - 1))
                nc.vector.tensor_copy(warm_scratch, dps)

            for ct in range(CT):
                rows = slice(ct * P, (ct + 1) * P)
                nc.sync.dma_start(out=vc_sb[:, ct, :], in_=vc_i[rows, :])
            for h in range(2):
                cols = slice(h * C, (h + 1) * C)
                for ct in range(CT):
                    rows = slice(ct * P, (ct + 1) * P)
                    nc.sync.dma_start(out=WakT_sb[:, ct, cols],
                                      in_=W_akT[rows, cols])
            for ct in range(CT):
                rows = slice(ct * P, (ct + 1) * P)
                for h in range(2):
                    cols = slice(h * 512, (h + 1) * 512)
                    nc.sync.dma_start(out=WmadT_sb[:, ct, cols],
                                      in_=W_madT[rows, cols])
            for ct in range(CT):
                rows = slice(ct * P, (ct + 1) * P)
                for h in range(2):
                    cols = slice(h * 512, (h + 1) * 512)
                    nc.sync.dma_start(out=Wgcn_sb[:, ct, cols],
                                      in_=W_gcn[rows, cols])

            if WARM0:
                warm(WARM0, "w0")

            # ======= S3: expz = exp(W_ak vc + b_ak), (k, l_loc) ============
            cs1 = pp.tile([P, KL], F32, tag="ps")
            with nc.named_scope("S3_expz"):
                for kt in range(KT):
                    ksl = slice(kt * P, (kt + 1) * P)
                    ps = pp.tile([P, KL], F32, tag="ps")
                    for cc in range(CT):
                        nc.tensor.matmul(ps, WakT_sb[:, cc, ksl],
                                         vc_sb[:, cc, :],
                                         start=(cc == 0), stop=(cc == CT - 1))
                    nc.scalar.activation(exp_sb[:, kt, :], ps, Exp,
                                         bias=b_ak_sb[:, kt:kt + 1], scale=1.0)
                    # lag-1: colsum of tile kt-1 while ACT evacuates tile kt
                    if kt > 0:
                        nc.tensor.matmul(cs1, onesm, exp_sb[:, kt - 1, :],
                                         start=(kt == 1), stop=False)
                nc.tensor.matmul(cs1, onesm, exp_sb[:, KT - 1, :],
                                 start=False, stop=True)
                nc.vector.reciprocal(recipL, cs1)

            # ---- load gathered vmrT into SBUF (per k-tile, so S4's first
            #      group chases the DMA stream) ----
            for kt in range(KT):
                rows = slice(kt * P, (kt + 1) * P)
                nc.sync.dma_start(out=vmrT_full[:, kt, :], in_=ag1_out[rows, :])

            # PE warm-keepers across the AG1 window
            if WARM1:
                warm(WARM1, "w1")

            # ======= S4 full-width (N=256 matmuls are ~2x as efficient as
            # N=128); S5 stays l-split so AG2a still triggers early ========
            def s4(h):
                if h:
                    return
                for m in range(CT):
                    msl = slice(m * P, (m + 1) * P)
                    ps = pp.tile([P, KL], F32, tag="ps", name=f"ps4_{m}")
                    for kt in range(KT):
                        nc.tensor.matmul(ps, vmrT_full[:, kt, msl],
                                         exp_sb[:, kt, :],
                                         start=(kt == 0), stop=(kt == KT - 1))
                    nc.vector.tensor_mul(vma_sb[:, m, :], ps, recipL)

            def s5(h, ag_in):
                lsl = slice(h * KLH, (h + 1) * KLH)
                for m in range(CT):
                    msl = slice(m * P, (m + 1) * P)
                    ps = pp.tile([P, KLH], F32, tag="ps", name=f"ps5_{h}_{m}")
                    for cc in range(CT):
                        nc.tensor.matmul(ps, WmadT_sb[:, cc, msl],
                                         vma_sb[:, cc, lsl],
                                         start=(cc == 0), stop=(cc == CT - 1))
                    nc.scalar.activation(vmad_i_sb[:, m, lsl], ps, Identity,
                                         bias=b_mad_sb[:, m:m + 1], scale=1.0)
                    nc.sync.dma_start(out=ag_in[msl, :],
                                      in_=vmad_i_sb[:, m, lsl])

            with nc.named_scope("S4S5a"):
                s4(0)
                s5(0, ag2a_in)
            # ======= AG2a: vmad chunk a (even global k-tiles) ==============
            nc.gpsimd.collective_compute(
                "AllGather", mybir.AluOpType.bypass, replica_groups=RG,
                ins=[ag2a_in.opt()], outs=[ag2a_out.opt()],
            )
            with nc.named_scope("S4S5b"):
                s4(1)
                s5(1, ag2b_in)
            # keep PE warm across the AG2 window
            if WARM2:
                warm(WARM2, "w2")
            # ======= AG2b: vmad chunk b (odd global k-tiles) ===============
            nc.gpsimd.collective_compute(
                "AllGather", mybir.AluOpType.bypass, replica_groups=RG,
                ins=[ag2b_in.opt()], outs=[ag2b_out.opt()],
            )
            for kt in range(KT):
                rows = slice(kt * P, (kt + 1) * P)
                nc.sync.dma_start(out=x_full[:, kt, :], in_=agx_out[rows, :])
            for s in range(NCORES):
                nc.sync.dma_start(
                    out=vmad_fullA[:, :, s, :],
                    in_=ag2a_out[s, :, :].rearrange("(t p) k -> p t k", p=P),
                )
            for s in range(NCORES):
                nc.sync.dma_start(
                    out=vmad_fullB[:, :, s, :],
                    in_=ag2b_out[s, :, :].rearrange("(t p) k -> p t k", p=P),
                )

            # ======= S7: expA = exp(vmad_full^T vmad_i) ====================
            # even k-tiles (chunk a) first, then odd (chunk b)
            cs2 = pp.tile([P, KL], F32, tag="ps")
            kt_order = [2 * s for s in range(NCORES)] + \
                       [2 * s + 1 for s in range(NCORES)]
            with nc.named_scope("S7_expA"):
                for idx, kt in enumerate(kt_order):
                    s, q = kt // 2, kt % 2
                    src = vmad_fullA if q == 0 else vmad_fullB
                    ps = pp.tile([P, KL], F32, tag="ps")
                    for cc in range(CT):
                        nc.tensor.matmul(ps, src[:, cc, s, :],
                                         vmad_i_sb[:, cc, :],
                                         start=(cc == 0), stop=(cc == CT - 1))
                    nc.scalar.activation(expA_sb[:, kt, :], ps, Exp, bias=negbias[:, 0:1], scale=1.0)
                    nc.tensor.matmul(cs2, onesm, expA_sb[:, kt, :],
                                     start=(idx == 0), stop=(idx == KT - 1))
                nc.vector.reciprocal(recipR, cs2)

            # ======= S8: out = (x^T expA) * recipR + b_gcn =================
            with nc.named_scope("S8_out"):
                for m in range(CT):
                    msl = slice(m * P, (m + 1) * P)
                    ps = pp.tile([P, KL], F32, tag="ps")
                    for idx, kt in enumerate(kt_order):
                        nc.tensor.matmul(ps, x_full[:, kt, msl],
                                         expA_sb[:, kt, :],
                                         start=(idx == 0), stop=(idx == KT - 1))
                    tmp = stage.tile([P, KL], F32, tag="s8tmp")
                    nc.vector.tensor_mul(tmp, ps, recipR)
                    o = stage.tile([P, KL], F32, tag="outstage")
                    nc.scalar.activation(o, tmp, Identity,
                                         bias=b_gcn_sb[:, m:m + 1], scale=1.0)
                    nc.sync.dma_start(out=out[m * P:(m + 1) * P, :], in_=o)

    nc.finalize()
    return nc


_NC_CACHE = None


def _get_nc():
    global _NC_CACHE
    if _NC_CACHE is None:
        _NC_CACHE = build()
    return _NC_CACHE


def _bf(a):
    return np.ascontiguousarray(a).astype(ml_dtypes.bfloat16)


def make_in_maps(inputs):
    """Shard + lay out the full inputs into the 8 per-core input maps."""
    vc0 = np.asarray(inputs["vc"])[0]
    vm0 = np.asarray(inputs["vm"])[0]
    shared = {
        "W_akT": _bf(np.asarray(inputs["W_ak"]).T),
        "W_cT": _bf(np.asarray(inputs["W_c"]).T),
        "W_madT": _bf(np.asarray(inputs["W_mad"]).T),
        "W_gcn": _bf(np.asarray(inputs["W_gcn"])),
        "b_ak_t": np.ascontiguousarray(
            np.asarray(inputs["b_ak"], np.float32).reshape(KT, P).T),
        "b_cB": np.ascontiguousarray(
            np.tile(np.asarray(inputs["b_c"], np.float32)[None, :], (P, 1))),
        "b_c_t": np.ascontiguousarray(
            np.asarray(inputs["b_c"], np.float32).reshape(CT, P).T),
        "b_mad_t": np.ascontiguousarray(
            np.asarray(inputs["b_mad"], np.float32).reshape(CT, P).T),
        "b_gcn_t": np.ascontiguousarray(
            np.asarray(inputs["b_gcn"], np.float32).reshape(CT, P).T),
    }
    in_maps = []
    for i in range(NCORES):
        cols = slice(i * KL, (i + 1) * KL)
        m = dict(shared)
        m["vc_i"] = _bf(vc0[:, cols])
        m["vm_i"] = _bf(vm0[:, cols])
        in_maps.append(m)
    return in_maps


def kernel(vc, vm, W_ak, b_ak, W_c, b_c, W_mad, b_mad, W_gcn, b_gcn):
    nc = _get_nc()
    in_maps = make_in_maps(dict(vc=vc, vm=vm, W_ak=W_ak, b_ak=b_ak, W_c=W_c,
                                b_c=b_c, W_mad=W_mad, b_mad=b_mad,
                                W_gcn=W_gcn, b_gcn=b_gcn))
    res = bass_utils.run_bass_kernel_spmd(nc, in_maps,
                                          core_ids=list(range(NCORES)))
    out = np.concatenate([np.asarray(res.results[i]["out"])
                          for i in range(NCORES)], axis=1)
    return out[None].astype(np.float32)


# revision 42
# speedup vs baseline: 1.0964x; 1.0964x over previous
"""Distributed Trainium2 Bass kernel for nn_AGCN (gnn_message_passing).

Reference computation (B=1, C=CHNN=1024, K=L=2048):
    vcw  = softmax_k(W_ak @ vc + b_ak)            # (K, L) assignment weights
    vmr  = relu(W_c @ vm + b_c)                   # (C, K)
    vma  = vmr @ vcw                              # (C, L)
    vmad = W_mad @ vma + b_mad                    # (C, L)
    A    = vmad^T @ vmad                          # (K, L) gram (symmetric)
    x    = vmr^T @ W_gcn + b_gcn                  # (K, C)
    out  = (softmax_rows(A) @ x)^T                # (C, L)

Distribution: position (node) sharding across 8 NeuronCores; core i owns
256 of the 2048 node columns.  Everything is local except three fp8
AllGathers on the serial ncfw stream:
  AG1  vmrT (2MB)  — right behind the kernel-entry barrier
  AGx  x = vmr^T W_gcn (2MB) — rides the stream's idle window between AG1
       and AG2, so it is effectively free; it eliminates the final W_gcn
       GEMM stage (out = (A_sm @ vmr^T) @ W_gcn == A_sm @ x, and the
       b_gcn fold is exact because softmax rows sum to 1)
  AG2  vmad shards in two k-chunks so the gram stage consumes chunk a
       while chunk b is in flight.

Matmuls are bf16/fp8 with fp32 PSUM accumulation (hw rel err ~1.9e-3 vs
the f32 reference; the A softmax is near-uniform so gram/fp8 noise
averages out).  Softmaxes skip max-subtraction (z in +-3.4, A in
[16.8, 17.2]); expA is scaled into fp8 range via a constant -12 bias
inside the ACT Exp (the colsum normalization auto-compensates).
Normalizations ride the PSUM-evacuation DVE ops; column sums are
computed on all partitions at once via ones-matrix matmuls with a lag-1
interleave so the PE never waits on ACT.  Dummy-matmul warm-keepers
(single long accumulation groups) bridge the collective windows to keep
the PE HAM clock at 2.4 GHz.  DMA emission order matters: loads gated on
collective semaphores are emitted after everything that must not queue
behind them.
"""

import numpy as np
import ml_dtypes

import concourse.bass as bass
import concourse.mybir as mybir
import concourse.tile as tile
from concourse import bacc
from concourse import bass_utils

P = 128            # partitions
C = 1024           # channels (8 tiles)
K = 2048           # nodes (16 tiles)
NCORES = 8
KL = K // NCORES   # 256 local node columns per core
KLH = KL // 2      # 128 (l/AG2 chunk width)
CT = C // P        # 8
KT = K // P        # 16
KLT = KL // P      # 2

BF = mybir.dt.bfloat16
F8 = mybir.dt.float8e4
F32 = mybir.dt.float32
RG = [list(range(NCORES))]

Exp = mybir.ActivationFunctionType.Exp
Identity = mybir.ActivationFunctionType.Identity

WARM0 = 100   # PE warm-keeper matmul counts (0 = disabled)
WARM1 = 240
WARM2 = 150
WARM3 = 60


def build():
    nc = bacc.Bacc("TRN2", target_bir_lowering=False, debug=False,
                   num_devices=NCORES)

    # ---- kernel I/O (per-core) ----
    vc_i = nc.dram_tensor("vc_i", [C, KL], BF, kind="ExternalInput").ap()
    vm_i = nc.dram_tensor("vm_i", [C, KL], BF, kind="ExternalInput").ap()
    W_akT = nc.dram_tensor("W_akT", [C, K], BF, kind="ExternalInput").ap()
    W_cT = nc.dram_tensor("W_cT", [C, C], BF, kind="ExternalInput").ap()
    W_madT = nc.dram_tensor("W_madT", [C, C], BF, kind="ExternalInput").ap()
    W_gcn = nc.dram_tensor("W_gcn", [C, C], BF, kind="ExternalInput").ap()
    b_ak_t = nc.dram_tensor("b_ak_t", [P, KT], F32, kind="ExternalInput").ap()
    b_cB = nc.dram_tensor("b_cB", [P, C], F32, kind="ExternalInput").ap()
    b_c_t = nc.dram_tensor("b_c_t", [P, CT], F32, kind="ExternalInput").ap()
    b_mad_t = nc.dram_tensor("b_mad_t", [P, CT], F32, kind="ExternalInput").ap()
    b_gcn_t = nc.dram_tensor("b_gcn_t", [P, CT], F32, kind="ExternalInput").ap()
    out = nc.dram_tensor("out", [C, KL], F32, kind="ExternalOutput").ap()

    with tile.TileContext(nc) as tc:
        with (
            tc.tile_pool(name="const", bufs=1) as const,
            tc.tile_pool(name="stage", bufs=4) as stage,
            tc.tile_pool(name="psum", bufs=8, space="PSUM") as pp,
            tc.tile_pool(name="dram", bufs=1, space="DRAM") as dram,
        ):
            # ---- persistent SBUF tensors ----
            vm_sb = const.tile([P, CT, KL], BF)       # vm   [p, ct, kl]
            vc_sb = const.tile([P, CT, KL], BF)
            WcT_sb = const.tile([P, CT, C], BF)
            WakT_sb = const.tile([P, CT, K], BF)
            WmadT_sb = const.tile([P, CT, C], BF)
            Wgcn_sb = const.tile([P, CT, C], BF)
            vmwT_full = const.tile([P, KT, C], F8)    # (W_mad vmr)^T[k, c]
            vmr_ck = const.tile([P, CT, KL], BF)      # vmr  [c, k_loc]
            # x = vmr^T W_gcn gathered in fp8; shares W_gcn's slot (W_gcn is
            # only needed by the early x GEMM)
            x_full = const.tile([P, KT, C], F8, tag="Wgcn_sb")
            # vmad_full by AG2 chunk: A = even global k-tiles, B = odd;
            # [p, ct, s, kl] = vmad[ct*128+p, s*256 + q*128 + kl]
            # chunk A shares the WakT slot (dead after S3, disjoint lifetime)
            vmad_fullA = const.tile([P, CT, NCORES, KLH], F8, tag="WakT_sb")
            vmad_fullB = const.tile([P, CT, NCORES, KLH], F8)
            exp_sb = const.tile([P, KT, KL], F8)      # expz then expA (reused)
            expA_sb = exp_sb
            vma_sb = const.tile([P, CT, KL], BF)
            vmad_i_sb = const.tile([P, CT, KL], F8)
            b_ak_sb = const.tile([P, KT], F32)
            b_cB_sb = const.tile([P, C], F32)
            b_c_t_sb = const.tile([P, CT], F32)
            b_mad_sb = const.tile([P, CT], F32)
            b_gcn_sb = const.tile([P, CT], F32)
            onesm = const.tile([P, P], F8)
            negbias = const.tile([P, 1], F32)         # -12.0 for scaled expA
            recipL = const.tile([P, KL], F32)
            recipR = const.tile([P, KL], F32)

            nc.any.memset(onesm, 1.0)
            nc.any.memset(negbias, -12.0)

            # ---- front input loads: S1's deps only ----
            nc.sync.dma_start(out=b_cB_sb, in_=b_cB)
            for ct in range(CT):
                rows = slice(ct * P, (ct + 1) * P)
                nc.sync.dma_start(out=vm_sb[:, ct, :], in_=vm_i[rows, :])
            for ct in range(CT):
                rows = slice(ct * P, (ct + 1) * P)
                for h in range(2):
                    cols = slice(h * 512, (h + 1) * 512)
                    nc.sync.dma_start(out=WcT_sb[:, ct, cols],
                                      in_=W_cT[rows, cols])
            for ct in range(CT):
                rows = slice(ct * P, (ct + 1) * P)
                for h in range(2):
                    cols = slice(h * 512, (h + 1) * 512)
                    nc.sync.dma_start(out=WmadT_sb[:, ct, cols],
                                      in_=W_madT[rows, cols])
            nc.sync.dma_start(out=b_c_t_sb, in_=b_c_t)
            nc.sync.dma_start(out=b_ak_sb, in_=b_ak_t)
            nc.sync.dma_start(out=b_mad_sb, in_=b_mad_t)
            nc.sync.dma_start(out=b_gcn_sb, in_=b_gcn_t)

            # ---- collective bounce buffers ----
            ag1_in = dram.tile([KL, C], F8)
            ag1_out = dram.tile([K, C], F8, addr_space="Shared")
            agx_in = dram.tile([KL, C], F8)
            agx_out = dram.tile([K, C], F8, addr_space="Shared")
            ag2a_in = dram.tile([C, KLH], F8)
            ag2a_out = dram.tile([NCORES, C, KLH], F8, addr_space="Shared")
            ag2b_in = dram.tile([C, KLH], F8)
            ag2b_out = dram.tile([NCORES, C, KLH], F8, addr_space="Shared")

            # ======= S1': vmr_i = relu(W_c vm_i + b_c), (c, k_loc) =========
            with nc.named_scope("S1p_vmr"):
                for m in range(CT):
                    msl = slice(m * P, (m + 1) * P)
                    ps = pp.tile([P, KL], F32, tag="ps", name=f"ps1p_{m}")
                    for cc in range(CT):
                        nc.tensor.matmul(ps, WcT_sb[:, cc, msl],
                                         vm_sb[:, cc, :],
                                         start=(cc == 0), stop=(cc == CT - 1))
                    nc.scalar.activation(vmr_ck[:, m, :], ps,
                                         mybir.ActivationFunctionType.Relu,
                                         bias=b_c_t_sb[:, m:m + 1], scale=1.0)

            # ======= S2a: vmwT_i = (W_mad vmr_i)^T, (k_loc, c) =============
            # so S4 emits vmad directly and the old S5 stage disappears:
            # (W_mad vmr expz) * recipL + b_mad == W_mad vma + b_mad
            with nc.named_scope("S2a_vmwT"):
                for kt in range(KLT):
                    ksl = slice(kt * P, (kt + 1) * P)
                    for n in range(2):
                        nsl = slice(n * 512, (n + 1) * 512)
                        ps = pp.tile([P, 512], F32, tag="ps",
                                     name=f"ps2a_{kt}_{n}")
                        for cc in range(CT):
                            nc.tensor.matmul(ps, vmr_ck[:, cc, ksl],
                                             WmadT_sb[:, cc, nsl],
                                             start=(cc == 0),
                                             stop=(cc == CT - 1))
                        ws = stage.tile([P, 512], F8, tag="s1relu")
                        nc.vector.tensor_copy(ws, ps)
                        nc.sync.dma_start(out=ag1_in[ksl, nsl], in_=ws)

            # ======= AG1: all-gather vmwT (single op — chunking halves the
            # per-op bus bandwidth and nothing can fill a second window) ====
            nc.gpsimd.collective_compute(
                "AllGather", mybir.AluOpType.bypass, replica_groups=RG,
                ins=[ag1_in.opt()], outs=[ag1_out.opt()],
            )

            # W_gcn loads ride after the AG1 bounce chain so they don't
            # delay its trigger; S2b (their consumer) runs well before AGx
            for ct in range(CT):
                rows = slice(ct * P, (ct + 1) * P)
                for h in range(2):
                    cols = slice(h * 512, (h + 1) * 512)
                    nc.sync.dma_start(out=Wgcn_sb[:, ct, cols],
                                      in_=W_gcn[rows, cols])

            # ======= S2b: x_i = vmr_i^T W_gcn, (k_loc, co) =================
            with nc.named_scope("S2b_x"):
                for kt in range(KLT):
                    ksl = slice(kt * P, (kt + 1) * P)
                    for n in range(2):
                        nsl = slice(n * 512, (n + 1) * 512)
                        ps = pp.tile([P, 512], F32, tag="ps",
                                     name=f"ps2b_{kt}_{n}")
                        for cc in range(CT):
                            nc.tensor.matmul(ps, vmr_ck[:, cc, ksl],
                                             Wgcn_sb[:, cc, nsl],
                                             start=(cc == 0),
                                             stop=(cc == CT - 1))
                        xs = stage.tile([P, 512], F8, tag="xstage")
                        nc.vector.tensor_copy(xs, ps)
                        nc.sync.dma_start(out=agx_in[ksl, nsl], in_=xs)

            # ======= AGx: all-gather x — rides the ncfw stream's idle
            # window between AG1 and AG2, so it is effectively free =========
            nc.gpsimd.collective_compute(
                "AllGather", mybir.AluOpType.bypass, replica_groups=RG,
                ins=[agx_in.opt()], outs=[agx_out.opt()],
            )

            # ---- remaining input loads (S3/S5 deps), after AG1's chain
            # (warm-keeper defined early so S1->S3 DMA wait stays warm)
            warm_scratch = const.tile([P, P], F32)

            def warm(n, label):
                dps = pp.tile([P, P], F32, tag="ps", name=f"warm_{label}")
                for i in range(n):
                    nc.tensor.matmul(dps, onesm, onesm,
                                     start=(i == 0), stop=(i == n - 1))
                nc.vector.tensor_copy(warm_scratch, dps)

            for ct in range(CT):
                rows = slice(ct * P, (ct + 1) * P)
                nc.sync.dma_start(out=vc_sb[:, ct, :], in_=vc_i[rows, :])
            for h in range(2):
                cols = slice(h * C, (h + 1) * C)
                for ct in range(CT):
                    rows = slice(ct * P, (ct + 1) * P)
                    nc.sync.dma_start(out=WakT_sb[:, ct, cols],
                                      in_=W_akT[rows, cols])
            for ct in range(CT):
                rows = slice(ct * P, (ct + 1) * P)
                for h in range(2):
                    cols = slice(h * 512, (h + 1) * 512)
                    nc.sync.dma_start(out=WmadT_sb[:, ct, cols],
                                      in_=W_madT[rows, cols])
            for ct in range(CT):
                rows = slice(ct * P, (ct + 1) * P)
                for h in range(2):
                    cols = slice(h * 512, (h + 1) * 512)
                    nc.sync.dma_start(out=Wgcn_sb[:, ct, cols],
                                      in_=W_gcn[rows, cols])

            if WARM0:
                warm(WARM0, "w0")

            # ======= S3: expz = exp(W_ak vc + b_ak), (k, l_loc) ============
            cs1 = pp.tile([P, KL], F32, tag="ps")
            with nc.named_scope("S3_expz"):
                for kt in range(KT):
                    ksl = slice(kt * P, (kt + 1) * P)
                    ps = pp.tile([P, KL], F32, tag="ps")
                    for cc in range(CT):
                        nc.tensor.matmul(ps, WakT_sb[:, cc, ksl],
                                         vc_sb[:, cc, :],
                                         start=(cc == 0), stop=(cc == CT - 1))
                    nc.scalar.activation(exp_sb[:, kt, :], ps, Exp,
                                         bias=b_ak_sb[:, kt:kt + 1], scale=1.0)
                    # lag-1: colsum of tile kt-1 while ACT evacuates tile kt
                    if kt > 0:
                        nc.tensor.matmul(cs1, onesm, exp_sb[:, kt - 1, :],
                                         start=(kt == 1), stop=False)
                nc.tensor.matmul(cs1, onesm, exp_sb[:, KT - 1, :],
                                 start=False, stop=True)
                nc.vector.reciprocal(recipL, cs1)

            # ---- load gathered vmrT into SBUF (per k-tile, so S4's first
            #      group chases the DMA stream) ----
            for kt in range(KT):
                rows = slice(kt * P, (kt + 1) * P)
                nc.sync.dma_start(out=vmwT_full[:, kt, :], in_=ag1_out[rows, :])

            # PE warm-keepers across the AG1 window
            if WARM1:
                warm(WARM1, "w1")

            # ======= S4: vmad_i = (vmw @ expz) * recipL + b_mad ===========
            with nc.named_scope("S4_vmad"):
                for m in range(CT):
                    msl = slice(m * P, (m + 1) * P)
                    ps = pp.tile([P, KL], F32, tag="ps", name=f"ps4_{m}")
                    for kt in range(KT):
                        nc.tensor.matmul(ps, vmwT_full[:, kt, msl],
                                         exp_sb[:, kt, :],
                                         start=(kt == 0), stop=(kt == KT - 1))
                    tmp = stage.tile([P, KL], F32, tag="s4tmp")
                    nc.vector.tensor_mul(tmp, ps, recipL)
                    nc.scalar.activation(vmad_i_sb[:, m, :], tmp, Identity,
                                         bias=b_mad_sb[:, m:m + 1], scale=1.0)
                    nc.sync.dma_start(out=ag2a_in[msl, :],
                                      in_=vmad_i_sb[:, m, :KLH])
            # ======= AG2a: vmad chunk a (even global k-tiles) ==============
            nc.gpsimd.collective_compute(
                "AllGather", mybir.AluOpType.bypass, replica_groups=RG,
                ins=[ag2a_in.opt()], outs=[ag2a_out.opt()],
            )
            for m in range(CT):
                msl = slice(m * P, (m + 1) * P)
                nc.sync.dma_start(out=ag2b_in[msl, :],
                                  in_=vmad_i_sb[:, m, KLH:])
            # keep PE warm across the AG2 window
            if WARM2:
                warm(WARM2, "w2")
            # ======= AG2b: vmad chunk b (odd global k-tiles) ===============
            nc.gpsimd.collective_compute(
                "AllGather", mybir.AluOpType.bypass, replica_groups=RG,
                ins=[ag2b_in.opt()], outs=[ag2b_out.opt()],
            )
            for kt in range(KT):
                rows = slice(kt * P, (kt + 1) * P)
                nc.sync.dma_start(out=x_full[:, kt, :], in_=agx_out[rows, :])
            for s in range(NCORES):
                nc.sync.dma_start(
                    out=vmad_fullA[:, :, s, :],
                    in_=ag2a_out[s, :, :].rearrange("(t p) k -> p t k", p=P),
                )
            for s in range(NCORES):
                nc.sync.dma_start(
                    out=vmad_fullB[:, :, s, :],
                    in_=ag2b_out[s, :, :].rearrange("(t p) k -> p t k", p=P),
                )

            # ======= S7: expA = exp(vmad_full^T vmad_i) ====================
            # even k-tiles (chunk a) first, then odd (chunk b)
            cs2 = pp.tile([P, KL], F32, tag="ps")
            kt_order = [2 * s for s in range(NCORES)] + \
                       [2 * s + 1 for s in range(NCORES)]
            with nc.named_scope("S7_expA"):
                for idx, kt in enumerate(kt_order):
                    s, q = kt // 2, kt % 2
                    src = vmad_fullA if q == 0 else vmad_fullB
                    ps = pp.tile([P, KL], F32, tag="ps")
                    for cc in range(CT):
                        nc.tensor.matmul(ps, src[:, cc, s, :],
                                         vmad_i_sb[:, cc, :],
                                         start=(cc == 0), stop=(cc == CT - 1))
                    nc.scalar.activation(expA_sb[:, kt, :], ps, Exp, bias=negbias[:, 0:1], scale=1.0)
                    nc.tensor.matmul(cs2, onesm, expA_sb[:, kt, :],
                                     start=(idx == 0), stop=(idx == KT - 1))
                nc.vector.reciprocal(recipR, cs2)

            # ======= S8: out = (x^T expA) * recipR + b_gcn =================
            with nc.named_scope("S8_out"):
                for m in range(CT):
                    msl = slice(m * P, (m + 1) * P)
                    ps = pp.tile([P, KL], F32, tag="ps")
                    for idx, kt in enumerate(kt_order):
                        nc.tensor.matmul(ps, x_full[:, kt, msl],
                                         expA_sb[:, kt, :],
                                         start=(idx == 0), stop=(idx == KT - 1))
                    tmp = stage.tile([P, KL], F32, tag="s8tmp")
                    nc.vector.tensor_mul(tmp, ps, recipR)
                    o = stage.tile([P, KL], F32, tag="outstage")
                    nc.scalar.activation(o, tmp, Identity,
                                         bias=b_gcn_sb[:, m:m + 1], scale=1.0)
                    nc.sync.dma_start(out=out[m * P:(m + 1) * P, :], in_=o)

    nc.finalize()
    return nc


_NC_CACHE = None


def _get_nc():
    global _NC_CACHE
    if _NC_CACHE is None:
        _NC_CACHE = build()
    return _NC_CACHE


def _bf(a):
    return np.ascontiguousarray(a).astype(ml_dtypes.bfloat16)


def make_in_maps(inputs):
    """Shard + lay out the full inputs into the 8 per-core input maps."""
    vc0 = np.asarray(inputs["vc"])[0]
    vm0 = np.asarray(inputs["vm"])[0]
    shared = {
        "W_akT": _bf(np.asarray(inputs["W_ak"]).T),
        "W_cT": _bf(np.asarray(inputs["W_c"]).T),
        "W_madT": _bf(np.asarray(inputs["W_mad"]).T),
        "W_gcn": _bf(np.asarray(inputs["W_gcn"])),
        "b_ak_t": np.ascontiguousarray(
            np.asarray(inputs["b_ak"], np.float32).reshape(KT, P).T),
        "b_cB": np.ascontiguousarray(
            np.tile(np.asarray(inputs["b_c"], np.float32)[None, :], (P, 1))),
        "b_c_t": np.ascontiguousarray(
            np.asarray(inputs["b_c"], np.float32).reshape(CT, P).T),
        "b_mad_t": np.ascontiguousarray(
            np.asarray(inputs["b_mad"], np.float32).reshape(CT, P).T),
        "b_gcn_t": np.ascontiguousarray(
            np.asarray(inputs["b_gcn"], np.float32).reshape(CT, P).T),
    }
    in_maps = []
    for i in range(NCORES):
        cols = slice(i * KL, (i + 1) * KL)
        m = dict(shared)
        m["vc_i"] = _bf(vc0[:, cols])
        m["vm_i"] = _bf(vm0[:, cols])
        in_maps.append(m)
    return in_maps


def kernel(vc, vm, W_ak, b_ak, W_c, b_c, W_mad, b_mad, W_gcn, b_gcn):
    nc = _get_nc()
    in_maps = make_in_maps(dict(vc=vc, vm=vm, W_ak=W_ak, b_ak=b_ak, W_c=W_c,
                                b_c=b_c, W_mad=W_mad, b_mad=b_mad,
                                W_gcn=W_gcn, b_gcn=b_gcn))
    res = bass_utils.run_bass_kernel_spmd(nc, in_maps,
                                          core_ids=list(range(NCORES)))
    out = np.concatenate([np.asarray(res.results[i]["out"])
                          for i in range(NCORES)], axis=1)
    return out[None].astype(np.float32)
